# revision 1
# baseline (speedup 1.0000x reference)
"""Multi-head causal attention (QKV proj + attention + out proj) on 8 TRN2
NeuronCores.

Sharding: 2-way data-parallel over batch x 4-way tensor-parallel over heads
(Megatron-style).  Core c handles batch c//4 and heads [4*(c%4), 4*(c%4)+4).
Each core computes its 4 heads' Q/K/V projections (column-parallel), the
attention for those heads, and a partial output projection (row-parallel).
The host sums the 4 TP partials per batch and adds the output bias.

Device layout notes (per core):
  - Q^T/K^T kept "d-major": [f=256 on partitions as 2 blocks of 128, S free].
    Weights are fed pre-transposed from the host so no on-chip transposes are
    needed; the softmax scale 1/sqrt(D) is folded into Wq/bq on the host.
  - scores^T[k, q] blocks [128, 512] = K_h^T-tile.T-matmul; exp on ACT;
    causal diagonal blocks multiplied by a binary mask (4 static tiles).
  - P^T @ [V|1] accumulated on PE per q-chunk: V carries a trailing ones
    column so the same matmul emits the softmax denominator as PSUM row D.
  - normalization: reciprocal of denom row, PE-broadcast to 64 partitions,
    single tensor_mult -> O^T; out-proj consumes O^T directly.  The chain is
    emitted one head late so the slow reciprocal never stalls the PE stream.
  - projections/out-proj matmuls run in float32r (full-rate fp32 mode);
    the attention matmuls (QK^T, PV) run in fp16 (full clock + fast weight
    loads; every operand is within fp16 range, measured rel err ~3e-4).
"""

import numpy as np
from contextlib import ExitStack

import concourse.bass as bass
import concourse.mybir as mybir
import concourse.tile as tile
from concourse import bacc
from concourse.bass import ds
from concourse.bass_utils import run_bass_kernel_spmd

B, S_FULL, E, H = 2, 2048, 1024, 16
D = E // H          # 64
NCORES = 8
TP = 4              # tensor-parallel ways (over heads)
HL = H // TP        # 4 local heads per core
F = HL * D          # 256 local projection width
P = 128
QCH = 512           # q-chunk / matmul moving-dim size
FP32 = mybir.dt.float32
F32R = mybir.dt.float32r
F16 = mybir.dt.float16
AF = mybir.ActivationFunctionType


def build(S=S_FULL, causal=True):
    ET = E // P          # 8 contraction tiles for projections
    NQ = S // QCH        # q chunks
    KT = S // P          # k tiles
    KPQ = QCH // P       # k tiles per q chunk (4)

    nc = bacc.Bacc()

    def din(name, shape, dt=FP32):
        return nc.declare_dram_parameter(name, shape, dt, isOutput=False)

    xqT = din("xqT", [E, S], F32R)
    xkT = din("xkT", [E, S], F32R)
    xvT = din("xvT", [E, S], F32R)
    wqT = din("wqT", [E, F], F32R)
    wkT = din("wkT", [E, F], F32R)
    wvT = din("wvT", [E, F], F32R)
    bq2 = din("bq2", [P, F // P])
    bk2 = din("bk2", [P, F // P])
    bvb = din("bvb", [P, F])
    woT = din("woT", [F, E], F32R)
    msk = din("msk", [P, KPQ, QCH], F16)
    outT = nc.declare_dram_parameter("outT", [E, S], FP32, isOutput=True)

    with ExitStack() as ctx:
        ctx.enter_context(
            nc.allow_low_precision(reason="float32r is the intended matmul input dtype")
        )
        tc = ctx.enter_context(tile.TileContext(nc))
        const = ctx.enter_context(tc.tile_pool(name="const", bufs=1))
        xp = ctx.enter_context(tc.tile_pool(name="xp", bufs=3))
        pex = ctx.enter_context(tc.tile_pool(name="pex", bufs=4))
        prn = ctx.enter_context(tc.tile_pool(name="prn", bufs=2))
        opool = ctx.enter_context(tc.tile_pool(name="op", bufs=3))
        pp = ctx.enter_context(tc.tile_pool(name="pp", bufs=2, space="PSUM"))
        psc = ctx.enter_context(tc.tile_pool(name="psc", bufs=2, space="PSUM"))
        po = ctx.enter_context(tc.tile_pool(name="po", bufs=3, space="PSUM"))
        pb = ctx.enter_context(tc.tile_pool(name="pb", bufs=1, space="PSUM"))

        # ---- constants / persistent tensors ----
        # masks first: the PE warm-up matmuls below depend only on this
        # small DMA, so the PE clock ramps while the big loads stream in.
        msk_sb = const.tile([P, KPQ, QCH], F16)
        nc.sync.dma_start(out=msk_sb, in_=msk[:, :, :])
        wq_sb = const.tile([P, ET, F], F32R)
        nc.sync.dma_start(out=wq_sb, in_=wqT[:, :].rearrange("(t p) f -> p t f", p=P))
        wk_sb = const.tile([P, ET, F], F32R)
        nc.sync.dma_start(out=wk_sb, in_=wkT[:, :].rearrange("(t p) f -> p t f", p=P))
        wv_sb = const.tile([P, ET, F], F32R)
        nc.sync.dma_start(out=wv_sb, in_=wvT[:, :].rearrange("(t p) f -> p t f", p=P))
        wo_sb = const.tile([P, F // P, E], F32R)
        nc.sync.dma_start(out=wo_sb, in_=woT[:, :].rearrange("(b p) e -> p b e", p=P))
        bq_sb = const.tile([P, F // P], FP32)
        nc.sync.dma_start(out=bq_sb, in_=bq2[:, :])
        bk_sb = const.tile([P, F // P], FP32)
        nc.sync.dma_start(out=bk_sb, in_=bk2[:, :])
        bvb_sb = const.tile([P, F], FP32)
        nc.sync.dma_start(out=bvb_sb, in_=bvb[:, :])
        # PE clock warm-up: ~28 back-to-back dummy matmuls (WAW-serialized on
        # one PSUM tile) keep the tensor engine busy through the HAM window
        # while the input DMAs stream, so real work starts at 2.4 GHz.
        wps = pb.tile([P, QCH], FP32, tag="bc")
        for _ in range(28):
            nc.tensor.matmul(
                wps, msk_sb[:, 0, 0:P], msk_sb[:, 0, :], start=True, stop=True
            )
        # memset cannot emit float32r (walrus ISA check); go via an FP32
        # scratch + ACT copy, which is a legal f32r producer.
        ones_f32 = const.tile([P, D], FP32)
        nc.vector.memset(ones_f32, 1.0)
        ones_sb = const.tile([P, D], F32R)
        nc.scalar.activation(ones_sb, ones_f32, AF.Copy)

        qT_sb = const.tile([P, F // P, S], F16)
        kT_sb = const.tile([P, F // P, S], F16)
        # V with a trailing ones column: AV matmul emits the softmax
        # denominator as PSUM row D for free.
        vo_sb = const.tile([P, KT, HL, D + 1], F16)
        nc.scalar.activation(
            vo_sb[:, :, :, D:D + 1],
            ones_f32[:, 0:KT * HL].rearrange("p (a b c) -> p a b c", a=KT, b=HL, c=1),
            AF.Copy,
        )
        oT_sb = const.tile([P, F // P, S], F32R)

        # ---- projections ----
        for j in range(NQ):
            for (xT, w_sb, b_sb, dst) in (
                (xqT, wq_sb, bq_sb, qT_sb),
                (xkT, wk_sb, bk_sb, kT_sb),
            ):
                xt = xp.tile([P, ET, QCH], F32R, tag="xt")
                nc.sync.dma_start(
                    out=xt,
                    in_=xT[:, :].rearrange("(t p) s -> p t s", p=P)[:, :, ds(j * QCH, QCH)],
                )
                for blk in range(F // P):
                    acc = pp.tile([P, QCH], FP32, tag="acc")
                    for et in range(ET):
                        nc.tensor.matmul(
                            acc,
                            w_sb[:, et, ds(blk * P, P)],
                            xt[:, et, :],
                            start=(et == 0),
                            stop=(et == ET - 1),
                        )
                    nc.vector.tensor_scalar_add(
                        dst[:, blk, ds(j * QCH, QCH)], acc, b_sb[:, blk:blk + 1]
                    )
            # V projection in s-major layout, written between the ones columns
            xt = xp.tile([P, ET, QCH], F32R, tag="xt")
            nc.sync.dma_start(
                out=xt,
                in_=xvT[:, :].rearrange("(t p) s -> p t s", p=P)[:, :, ds(j * QCH, QCH)],
            )
            for sl in range(QCH // P):
                st = j * (QCH // P) + sl
                acc = pp.tile([P, QCH], FP32, tag="acc")
                for et in range(ET):
                    nc.tensor.matmul(
                        acc[:, 0:F],
                        xt[:, et, ds(sl * P, P)],
                        wv_sb[:, et, :],
                        start=(et == 0),
                        stop=(et == ET - 1),
                    )
                for h in range(HL):
                    nc.vector.tensor_add(
                        vo_sb[:, st, h, 0:D],
                        acc[:, ds(h * D, D)],
                        bvb_sb[:, ds(h * D, D)],
                    )

        # ---- attention ----
        # Normalization of head (j,h) is emitted AFTER the next head's
        # matmul block: the slow single-partition reciprocal (≈3.4us on DVE)
        # otherwise stalls the in-order PE stream right before the bc
        # broadcast matmul and re-throttles the PE clock.
        def emit_normalize(j, h, po_t):
            blkh = h // 2
            doff = (h % 2) * D
            rc = prn.tile([P, QCH], F32R, tag="rc")
            nc.vector.reciprocal(rc[D:D + 1, :], po_t[D:D + 1, :])
            bc = pb.tile([P, QCH], FP32, tag="bc")
            nc.tensor.matmul(
                bc[0:D, :],
                ones_sb[D:D + 1, :],
                rc[D:D + 1, :],
                start=True,
                stop=True,
            )
            bcs = prn.tile([P, QCH], FP32, tag="bcs")
            nc.scalar.activation(bcs[0:D, :], bc[0:D, :], AF.Copy)
            nc.vector.tensor_mul(
                oT_sb[doff:doff + D, blkh, ds(j * QCH, QCH)],
                po_t[0:D, :],
                bcs[0:D, :],
            )

        pending = None
        for j in range(NQ):
            for h in range(HL):
                blkh = h // 2
                doff = (h % 2) * D          # partition offset of head h in blk

                nkt = KPQ * (j + 1) if causal else KT
                po_t = po.tile([P, QCH], FP32, tag="po")
                for kt in range(nkt):
                    sc = psc.tile([P, QCH], FP32, tag="sc")
                    nc.tensor.matmul(
                        sc,
                        kT_sb[doff:doff + D, blkh, ds(kt * P, P)],
                        qT_sb[doff:doff + D, blkh, ds(j * QCH, QCH)],
                        start=True,
                        stop=True,
                    )
                    pt = pex.tile([P, QCH], F16, tag="pt")
                    nc.scalar.activation(pt, sc, AF.Exp)
                    if causal and kt >= KPQ * j:
                        t = kt - KPQ * j
                        nc.vector.tensor_mul(pt, pt, msk_sb[:, t, :])
                    nc.tensor.matmul(
                        po_t[0:D + 1, :],
                        vo_sb[:, kt, h, :],
                        pt,
                        start=(kt == 0),
                        stop=(kt == nkt - 1),
                    )
                if pending is not None:
                    emit_normalize(*pending)
                pending = (j, h, po_t)
        emit_normalize(*pending)

        # ---- output projection (partial over local heads) ----
        for eb in range(E // P):
            for j in range(NQ):
                acc = pp.tile([P, QCH], FP32, tag="acc")
                for fb in range(F // P):
                    nc.tensor.matmul(
                        acc,
                        wo_sb[:, fb, ds(eb * P, P)],
                        oT_sb[:, fb, ds(j * QCH, QCH)],
                        start=(fb == 0),
                        stop=(fb == F // P - 1),
                    )
                ot = opool.tile([P, QCH], FP32, tag="ot")
                nc.vector.tensor_copy(ot, acc)
                nc.sync.dma_start(
                    out=outT[ds(eb * P, P), ds(j * QCH, QCH)], in_=ot
                )

    nc.compile()
    return nc


def make_masks(S=S_FULL):
    KPQ = QCH // P
    m = np.zeros((P, KPQ, QCH), np.float32)
    for t in range(KPQ):
        kk = np.arange(P)[:, None]
        qq = np.arange(QCH)[None, :]
        m[:, t, :] = (qq >= kk + P * t).astype(np.float32)
    return m


def make_in_maps(query, key, value, Wq, bq, Wk, bk, Wv, bv, Wo, bo, S=S_FULL):
    scale = float(D) ** -0.5
    q = np.asarray(query, np.float32)
    k = np.asarray(key, np.float32)
    v = np.asarray(value, np.float32)
    Wq = np.asarray(Wq, np.float32)
    Wk = np.asarray(Wk, np.float32)
    Wv = np.asarray(Wv, np.float32)
    Wo = np.asarray(Wo, np.float32)
    bq = np.asarray(bq, np.float32)
    bk = np.asarray(bk, np.float32)
    bv = np.asarray(bv, np.float32)
    masks = make_masks(S)
    in_maps = []
    for c in range(NCORES):
        b, tp = divmod(c, TP)
        rows = slice(tp * F, (tp + 1) * F)
        in_maps.append({
            "xqT": np.ascontiguousarray(q[b].T),
            "xkT": np.ascontiguousarray(k[b].T),
            "xvT": np.ascontiguousarray(v[b].T),
            "wqT": np.ascontiguousarray((Wq[rows] * scale).T),
            "wkT": np.ascontiguousarray(Wk[rows].T),
            "wvT": np.ascontiguousarray(Wv[rows].T),
            "bq2": np.ascontiguousarray((bq[rows] * scale).reshape(F // P, P).T),
            "bk2": np.ascontiguousarray(bk[rows].reshape(F // P, P).T),
            "bvb": np.ascontiguousarray(np.broadcast_to(bv[rows], (P, F))),
            "woT": np.ascontiguousarray(Wo[:, rows].T),
            "msk": masks.astype(np.float16),
        })
    return in_maps


_CACHE = {}


def _get_nc(causal):
    if causal not in _CACHE:
        _CACHE[causal] = build(S_FULL, causal)
    return _CACHE[causal]


def kernel(query, key, value, Wq, bq, Wk, bk, Wv, bv, Wo, bo, is_causal):
    causal = bool(int(np.asarray(is_causal)))
    nc = _get_nc(causal)
    in_maps = make_in_maps(query, key, value, Wq, bq, Wk, bk, Wv, bv, Wo, bo)
    res = run_bass_kernel_spmd(nc, in_maps, core_ids=list(range(NCORES)))
    out = np.zeros((B, S_FULL, E), np.float32)
    for c in range(NCORES):
        b, tp = divmod(c, TP)
        out[b] += res.results[c]["outT"].T
    out += np.asarray(bo, np.float32)
    return out



# revision 20
# speedup vs baseline: 1.3253x; 1.3253x over previous
"""Multi-head causal attention (QKV proj + attention + out proj) on 8 TRN2
NeuronCores.

Sharding: 2-way data-parallel over batch x 4-way tensor-parallel over heads
(Megatron-style).  Core c handles batch c//4 and heads [4*(c%4), 4*(c%4)+4).
Each core computes its 4 heads' Q/K/V projections (column-parallel), the
attention for those heads, and a partial output projection (row-parallel).
The host sums the 4 TP partials per batch and adds the output bias.

v2 design notes (vs the 333us baseline, which was PE-bound at 1.2 GHz):
  - everything fp16 on the wire: x, weights, output partials are cast on the
    host, halving HBM traffic (30MB -> ~15MB per core).  Host also
    pre-swizzles x/weights into partition-major layout so every input DMA is
    a contiguous per-partition stream.
  - exp batching: scores for 4 k-tiles accumulate into one [128, 2048] PSUM
    region (4 banks) and are exp'd by ONE activation instruction --
    (2048+352)/1.2 = 2.0us per 4 tiles vs 4x720ns.  ACT is the attention
    pace-setter so this matters.
  - normalization: reciprocal_approx_fast (custom DVE op, ~670ns for [1,512])
    replaces the 3355ns iterative reciprocal; the chain is emitted one head
    late so it never blocks the PE stream.
  - software-pipelined emission: the engines are in-order, so the PE
    instruction stream interleaves attention groups of chunk j with
    projection chains of chunk j+1 and out-projection blocks of chunk j-1
    ("filler" units).  The PE never sits on an exp dependency and the HAM
    clock gate stays at 8/8 (2.4 GHz).
"""

import numpy as np
from collections import deque
from contextlib import ExitStack

import concourse.bass as bass
import concourse.mybir as mybir
import concourse.tile as tile
from concourse import bacc
from concourse.bass import ds
from concourse.bass_utils import run_bass_kernel_spmd

B, S_FULL, E, H = 2, 2048, 1024, 16
D = E // H          # 64
NCORES = 8
TP = 4              # tensor-parallel ways (over heads)
HL = H // TP        # 4 local heads per core
F = HL * D          # 256 local projection width
P = 128
QCH = 512           # q-chunk / matmul moving-dim size
GRP = 4             # k-tiles exp'd per activation instruction
FP32 = mybir.dt.float32
F32R = mybir.dt.float32r
F16 = mybir.dt.float16
AF = mybir.ActivationFunctionType


def build(S=S_FULL, causal=True, debug=False):
    ET = E // P          # 8 contraction tiles for projections
    NQ = S // QCH        # 4 q chunks
    KT = S // P          # 16 k tiles
    KPQ = QCH // P       # 4 k tiles per q chunk

    nc = bacc.Bacc()

    def din(name, shape, dt=F16):
        return nc.declare_dram_parameter(name, shape, dt, isOutput=False)

    # host pre-swizzled, all fp16 (see make_in_maps)
    xq4 = din("xq4", [NQ, P, ET, QCH])
    xk4 = din("xk4", [NQ, P, ET, QCH])
    xv4 = din("xv4", [NQ, P, ET, QCH])
    wq3 = din("wq3", [P, ET, F])
    wk3 = din("wk3", [P, ET, F])
    wv3 = din("wv3", [P, ET, F])
    wo3 = din("wo3", [P, F // P, E])
    bcat = din("bcat", [P, 2 + 2 + F], FP32)   # bq2 | bk2 | bvb
    msk = din("msk", [P, KPQ, QCH])
    outT = nc.declare_dram_parameter("outT", [E, S], F16, isOutput=True)
    if debug:
        dbg_qT = nc.declare_dram_parameter("dbg_qT", [P, F // P, S], F16, isOutput=True)
        dbg_kT = nc.declare_dram_parameter("dbg_kT", [P, F // P, S], F16, isOutput=True)
        dbg_vo = nc.declare_dram_parameter("dbg_vo", [P, KT, HL, D + 1], F16, isOutput=True)
        dbg_oT = nc.declare_dram_parameter("dbg_oT", [P, F // P, S], F16, isOutput=True)
        dbg_po = nc.declare_dram_parameter("dbg_po", [P, HL, QCH], FP32, isOutput=True)
        dbg_bc = nc.declare_dram_parameter("dbg_bc", [D, HL, QCH], FP32, isOutput=True)

    with ExitStack() as ctx:
        ctx.enter_context(
            nc.allow_low_precision(reason="fp16 matmuls are the design point")
        )
        tc = ctx.enter_context(tile.TileContext(nc))
        const = ctx.enter_context(tc.tile_pool(name="const", bufs=1))
        xp = ctx.enter_context(tc.tile_pool(name="xp", bufs=2))  # 2 bufs x 3 tags
        ptp = ctx.enter_context(tc.tile_pool(name="ptp", bufs=2))
        rcp = ctx.enter_context(tc.tile_pool(name="rcp", bufs=2))
        bcsp = ctx.enter_context(tc.tile_pool(name="bcsp", bufs=2))
        otp = ctx.enter_context(tc.tile_pool(name="otp", bufs=3))
        # PSUM: sc 4 banks + acc 2 + po 2 = 8
        scp = ctx.enter_context(tc.tile_pool(name="scp", bufs=1, space="PSUM"))
        accp = ctx.enter_context(tc.tile_pool(name="accp", bufs=2, space="PSUM"))
        pop = ctx.enter_context(tc.tile_pool(name="pop", bufs=2, space="PSUM"))

        # ---- constants / persistent tensors ----
        # masks first: the PE warm-up matmuls depend only on this small DMA,
        # so the PE clock ramps while the big loads stream in.
        msk_sb = const.tile([P, KPQ, QCH], F16)
        nc.sync.dma_start(out=msk_sb, in_=msk[:, :, :])
        wq_sb = const.tile([P, ET, F], F16)
        nc.sync.dma_start(out=wq_sb, in_=wq3[:, :, :])
        wk_sb = const.tile([P, ET, F], F16)
        nc.sync.dma_start(out=wk_sb, in_=wk3[:, :, :])
        wv_sb = const.tile([P, ET, F], F16)
        nc.sync.dma_start(out=wv_sb, in_=wv3[:, :, :])
        wo_sb = const.tile([P, F // P, E], F16)
        nc.sync.dma_start(out=wo_sb, in_=wo3[:, :, :])
        bcat_sb = const.tile([P, 2 + 2 + F], FP32)
        nc.sync.dma_start(out=bcat_sb, in_=bcat[:, :])
        bq_sb = bcat_sb[:, 0:2]
        bk_sb = bcat_sb[:, 2:4]
        bvb_sb = bcat_sb[:, 4:4 + F]

        # PE clock warm-up: back-to-back dummy matmuls (WAW-serialized on the
        # acc pool) keep the tensor engine busy through the HAM window while
        # the input DMAs stream, so real work starts at 2.4 GHz.
        for _ in range(14):
            wps = accp.tile([P, QCH], FP32, tag="acc")
            nc.tensor.matmul(
                wps, msk_sb[:, 0, 0:P], msk_sb[:, 0, :], start=True, stop=True
            )

        ones_f32 = const.tile([P, D], FP32)
        nc.vector.memset(ones_f32, 1.0)
        ones_r = const.tile([P, D], F32R)
        nc.scalar.activation(ones_r, ones_f32, AF.Copy)

        qT_sb = const.tile([P, F // P, S], F16)
        kT_sb = const.tile([P, F // P, S], F16)
        # V with a trailing ones column: the PV matmul emits the softmax
        # denominator as PSUM row D for free.
        vo_sb = const.tile([P, KT, HL, D + 1], F16)
        nc.scalar.activation(
            vo_sb[:, :, :, D:D + 1],
            ones_f32[:, 0:KT * HL].rearrange("p (a b c) -> p a b c", a=KT, b=HL, c=1),
            AF.Copy,
        )
        oT_sb = const.tile([P, F // P, S], F16)

        # x chunk DMAs (j-granular); emitted early and prefetched one chunk
        # ahead by the main loop.
        x_tiles = {}

        def emit_x_dma(j):
            for name, src in (("q", xq4), ("k", xk4), ("v", xv4)):
                t = xp.tile([P, ET, QCH], F16, tag=f"x{name}")
                nc.sync.dma_start(out=t, in_=src[j])
                x_tiles[(name, j)] = t

        # ---- projection / out-projection unit generators (PE fillers) ----
        def proj_qk_unit(j, which, blk):
            xt = x_tiles[(which, j)]
            w_sb = wq_sb if which == "q" else wk_sb
            b_sb = bq_sb if which == "q" else bk_sb
            dst = qT_sb if which == "q" else kT_sb
            acc = accp.tile([P, QCH], FP32, tag="acc")
            for et in range(ET):
                nc.tensor.matmul(
                    acc,
                    w_sb[:, et, ds(blk * P, P)],
                    xt[:, et, :],
                    start=(et == 0),
                    stop=(et == ET - 1),
                )
            nc.vector.tensor_scalar_add(
                dst[:, blk, ds(j * QCH, QCH)], acc, b_sb[:, blk:blk + 1]
            )
            return 1750

        def proj_v_unit(j, sl):
            xt = x_tiles[("v", j)]
            st = j * KPQ + sl
            acc = accp.tile([P, QCH], FP32, tag="acc")
            for et in range(ET):
                nc.tensor.matmul(
                    acc[:, 0:F],
                    xt[:, et, ds(sl * P, P)],
                    wv_sb[:, et, :],
                    start=(et == 0),
                    stop=(et == ET - 1),
                )
            nc.vector.tensor_add(
                vo_sb[:, st, :, 0:D],
                acc[:, 0:F].rearrange("p (h d) -> p h d", h=HL),
                bvb_sb.rearrange("p (h d) -> p h d", h=HL),
            )
            return 900

        def outproj_unit(j, eb):
            acc = accp.tile([P, QCH], FP32, tag="acc")
            for fb in range(F // P):
                nc.tensor.matmul(
                    acc,
                    wo_sb[:, fb, ds(eb * P, P)],
                    oT_sb[:, fb, ds(j * QCH, QCH)],
                    start=(fb == 0),
                    stop=(fb == F // P - 1),
                )
            ot = otp.tile([P, QCH], F16, tag="ot")
            if eb % 2 == 0:
                nc.vector.tensor_copy(ot, acc)
            else:
                nc.scalar.activation(ot, acc, AF.Copy)
            nc.sync.dma_start(out=outT[ds(eb * P, P), ds(j * QCH, QCH)], in_=ot)
            return 500

        fillers = deque()

        def do_filler(budget):
            while budget > 0 and fillers:
                budget -= fillers.popleft()()

        def drain_fillers():
            while fillers:
                fillers.popleft()()

        def push_proj(j):
            for blk in range(F // P):
                fillers.append(lambda j=j, b=blk: proj_qk_unit(j, "q", b))
                fillers.append(lambda j=j, b=blk: proj_qk_unit(j, "k", b))
            for sl in range(KPQ):
                fillers.append(lambda j=j, s=sl: proj_v_unit(j, s))

        def push_outproj(j):
            for eb in range(E // P):
                fillers.append(lambda j=j, e=eb: outproj_unit(j, e))

        if debug:
            dbg_po_sb = const.tile([P, HL, QCH], FP32)
            dbg_bc_sb = const.tile([D, HL, QCH], FP32)

        # ---- normalization (emitted one head late) ----
        def emit_norm(j, h, po_t):
            blkh = h // 2
            doff = (h % 2) * D
            # 1/denom via ln -> broadcast -> exp(-x): all on ACT (exp and ln
            # share one table set), leaving DVE with just the final multiply.
            lg = rcp.tile([1, QCH], F32R, tag="lg")
            nc.scalar.activation(lg, po_t[D:D + 1, :], AF.Ln)
            bc = accp.tile([P, QCH], FP32, tag="acc")
            nc.tensor.matmul(
                bc[0:D, :],
                ones_r[0:1, :],
                lg,
                start=True,
                stop=True,
            )
            bcs = bcsp.tile([D, QCH], F16, tag="bcs")
            nc.scalar.activation(bcs, bc[0:D, :], AF.Exp, scale=-1.0)
            if debug and j == 0:
                nc.vector.tensor_copy(dbg_po_sb[:, h, :], po_t)
                nc.vector.tensor_copy(dbg_bc_sb[:, h, :], bc[0:D, :])
            nc.vector.tensor_mul(
                oT_sb[doff:doff + D, blkh, ds(j * QCH, QCH)],
                po_t[0:D, :],
                bcs,
            )

        # ---- main emission loop ----
        emit_x_dma(0)
        push_proj(0)
        drain_fillers()          # projections for chunk 0 up front

        pending = None
        for j in range(NQ):
            if j + 1 < NQ:
                emit_x_dma(j + 1)
                push_proj(j + 1)
            for h in range(HL):
                blkh = h // 2
                doff = (h % 2) * D
                nkt = KPQ * (j + 1) if causal else KT
                ngrp = (nkt + GRP - 1) // GRP
                po_t = pop.tile([P, QCH], FP32, tag="po")
                for g in range(ngrp):
                    kts = list(range(g * GRP, min(nkt, (g + 1) * GRP)))
                    nk = len(kts)
                    sc = scp.tile([P, GRP, QCH], FP32, tag="sc")
                    for i, kt in enumerate(kts):
                        nc.tensor.matmul(
                            sc[:, i, :],
                            kT_sb[doff:doff + D, blkh, ds(kt * P, P)],
                            qT_sb[doff:doff + D, blkh, ds(j * QCH, QCH)],
                            start=True,
                            stop=True,
                        )
                    pt = ptp.tile([P, GRP, QCH], F16, tag="pt")
                    nc.scalar.activation(pt[:, 0:nk, :], sc[:, 0:nk, :], AF.Exp)
                    if causal and kts[-1] >= KPQ * j:
                        # diagonal group: zero the upper-triangular part
                        nc.vector.tensor_mul(
                            pt[:, 0:nk, :], pt[:, 0:nk, :], msk_sb[:, 0:nk, :]
                        )
                    do_filler(1400)
                    for i, kt in enumerate(kts):
                        nc.tensor.matmul(
                            po_t[0:D + 1, :],
                            vo_sb[:, kt, h, :],
                            pt[:, i, :],
                            start=(kt == 0),
                            stop=(kt == nkt - 1),
                        )
                if pending is not None:
                    pj, ph, ppo = pending
                    emit_norm(pj, ph, ppo)
                    if ph == HL - 1:
                        push_outproj(pj)
                pending = (j, h, po_t)
            # chunk boundary: everything for chunk j+1's attention must be
            # emitted before its first scores matmul.
            drain_fillers()
        emit_norm(*pending)
        push_outproj(NQ - 1)
        drain_fillers()
        if debug:
            nc.sync.dma_start(out=dbg_qT[:, :, :], in_=qT_sb)
            nc.sync.dma_start(out=dbg_kT[:, :, :], in_=kT_sb)
            nc.sync.dma_start(out=dbg_vo[:, :, :, :], in_=vo_sb)
            nc.sync.dma_start(out=dbg_oT[:, :, :], in_=oT_sb)
            nc.sync.dma_start(out=dbg_po[:, :, :], in_=dbg_po_sb)
            nc.sync.dma_start(out=dbg_bc[:, :, :], in_=dbg_bc_sb)

    nc.compile()
    return nc


def make_masks(S=S_FULL):
    KPQ = QCH // P
    m = np.zeros((P, KPQ, QCH), np.float32)
    for t in range(KPQ):
        kk = np.arange(P)[:, None]
        qq = np.arange(QCH)[None, :]
        m[:, t, :] = (qq >= kk + P * t).astype(np.float32)
    return m


def make_in_maps(query, key, value, Wq, bq, Wk, bk, Wv, bv, Wo, bo, S=S_FULL):
    scale = float(D) ** -0.5
    ET = E // P
    NQ = S // QCH
    q = np.asarray(query, np.float32)
    k = np.asarray(key, np.float32)
    v = np.asarray(value, np.float32)
    Wq = np.asarray(Wq, np.float32)
    Wk = np.asarray(Wk, np.float32)
    Wv = np.asarray(Wv, np.float32)
    Wo = np.asarray(Wo, np.float32)
    bq = np.asarray(bq, np.float32)
    bk = np.asarray(bk, np.float32)
    bv = np.asarray(bv, np.float32)

    def xswiz(xT):
        # [E, S] -> [NQ, P, ET, QCH]: contiguous per-partition DMA streams
        return np.ascontiguousarray(
            xT.reshape(ET, P, NQ, QCH).transpose(2, 1, 0, 3).astype(np.float16)
        )

    def wswiz(wT):
        # [E, F] -> [P, ET, F]
        return np.ascontiguousarray(
            wT.reshape(ET, P, F).transpose(1, 0, 2).astype(np.float16)
        )

    masks = make_masks(S).astype(np.float16)
    in_maps = []
    for c in range(NCORES):
        b, tp = divmod(c, TP)
        rows = slice(tp * F, (tp + 1) * F)
        bq2 = (bq[rows] * scale).reshape(F // P, P).T        # [P, 2]
        bk2 = bk[rows].reshape(F // P, P).T                  # [P, 2]
        bvb = np.broadcast_to(bv[rows], (P, F))              # [P, F]
        bcat = np.concatenate([bq2, bk2, bvb], axis=1).astype(np.float32)
        woT = Wo[:, rows].T                                  # [F, E]
        wo3 = woT.reshape(F // P, P, E).transpose(1, 0, 2).astype(np.float16)
        in_maps.append({
            "xq4": xswiz(q[b].T),
            "xk4": xswiz(k[b].T),
            "xv4": xswiz(v[b].T),
            "wq3": wswiz((Wq[rows] * scale).T),
            "wk3": wswiz(Wk[rows].T),
            "wv3": wswiz(Wv[rows].T),
            "wo3": np.ascontiguousarray(wo3),
            "bcat": np.ascontiguousarray(bcat),
            "msk": masks,
        })
    return in_maps


_CACHE = {}


def _get_nc(causal):
    if causal not in _CACHE:
        _CACHE[causal] = build(S_FULL, causal)
    return _CACHE[causal]


def kernel(query, key, value, Wq, bq, Wk, bk, Wv, bv, Wo, bo, is_causal):
    causal = bool(int(np.asarray(is_causal)))
    nc = _get_nc(causal)
    in_maps = make_in_maps(query, key, value, Wq, bq, Wk, bk, Wv, bv, Wo, bo)
    res = run_bass_kernel_spmd(nc, in_maps, core_ids=list(range(NCORES)))
    out = np.zeros((B, S_FULL, E), np.float32)
    for c in range(NCORES):
        b, tp = divmod(c, TP)
        out[b] += res.results[c]["outT"].T.astype(np.float32)
    out += np.asarray(bo, np.float32)
    return out


# revision 26
# speedup vs baseline: 1.4203x; 1.0717x over previous
"""Multi-head causal attention (QKV proj + attention + out proj) on 8 TRN2
NeuronCores.

Sharding: 2-way data-parallel over batch x 4-way tensor-parallel over heads
(Megatron-style).  Core c handles batch c//4 and heads [4*(c%4), 4*(c%4)+4).
Each core computes its 4 heads' Q/K/V projections (column-parallel), the
attention for those heads, and a partial output projection (row-parallel).
The host sums the 4 TP partials per batch and adds the output bias.

v2 design notes (vs the 333us baseline, which was PE-bound at 1.2 GHz):
  - everything fp16 on the wire: x, weights, output partials are cast on the
    host, halving HBM traffic (30MB -> ~15MB per core).  Host also
    pre-swizzles x/weights into partition-major layout so every input DMA is
    a contiguous per-partition stream.
  - exp batching: scores for 4 k-tiles accumulate into one [128, 2048] PSUM
    region (4 banks) and are exp'd by ONE activation instruction --
    (2048+352)/1.2 = 2.0us per 4 tiles vs 4x720ns.  ACT is the attention
    pace-setter so this matters.
  - normalization: reciprocal_approx_fast (custom DVE op, ~670ns for [1,512])
    replaces the 3355ns iterative reciprocal; the chain is emitted one head
    late so it never blocks the PE stream.
  - software-pipelined emission: the engines are in-order, so the PE
    instruction stream interleaves attention groups of chunk j with
    projection chains of chunk j+1 and out-projection blocks of chunk j-1
    ("filler" units).  The PE never sits on an exp dependency and the HAM
    clock gate stays at 8/8 (2.4 GHz).
"""

import numpy as np
from collections import deque
from contextlib import ExitStack

import concourse.bass as bass
import concourse.mybir as mybir
import concourse.tile as tile
from concourse import bacc
from concourse.bass import ds
from concourse.bass_utils import run_bass_kernel_spmd

B, S_FULL, E, H = 2, 2048, 1024, 16
D = E // H          # 64
NCORES = 8
TP = 4              # tensor-parallel ways (over heads)
HL = H // TP        # 4 local heads per core
F = HL * D          # 256 local projection width
P = 128
QCH = 512           # q-chunk / matmul moving-dim size
GRP = 4             # k-tiles exp'd per activation instruction
FP32 = mybir.dt.float32
F32R = mybir.dt.float32r
F16 = mybir.dt.float16
AF = mybir.ActivationFunctionType


def build(S=S_FULL, causal=True, debug=False):
    ET = E // P          # 8 contraction tiles for projections
    NQ = S // QCH        # 4 q chunks
    KT = S // P          # 16 k tiles
    KPQ = QCH // P       # 4 k tiles per q chunk

    nc = bacc.Bacc()

    def din(name, shape, dt=F16):
        return nc.declare_dram_parameter(name, shape, dt, isOutput=False)

    # host pre-swizzled, all fp16 (see make_in_maps)
    xq4 = din("xq4", [NQ, P, ET, QCH])
    xk4 = din("xk4", [NQ, P, ET, QCH])
    xv4 = din("xv4", [NQ, P, ET, QCH])
    wq3 = din("wq3", [P, ET, F])
    wk3 = din("wk3", [P, ET, F])
    wv3 = din("wv3", [P, ET, F])
    wo3 = din("wo3", [P, F // P, E])
    bcat = din("bcat", [P, 2 + 2 + F], FP32)   # bq2 | bk2 | bvb
    msk = din("msk", [P, KPQ, QCH])
    outT = nc.declare_dram_parameter("outT", [E, S], F16, isOutput=True)
    if debug:
        dbg_qT = nc.declare_dram_parameter("dbg_qT", [P, F // P, S], F16, isOutput=True)
        dbg_kT = nc.declare_dram_parameter("dbg_kT", [P, F // P, S], F16, isOutput=True)
        dbg_vo = nc.declare_dram_parameter("dbg_vo", [P, KT, HL, D + 1], F16, isOutput=True)
        dbg_oT = nc.declare_dram_parameter("dbg_oT", [P, F // P, S], F16, isOutput=True)
        dbg_po = nc.declare_dram_parameter("dbg_po", [P, HL, QCH], FP32, isOutput=True)
        dbg_bc = nc.declare_dram_parameter("dbg_bc", [D, HL, QCH], FP32, isOutput=True)

    with ExitStack() as ctx:
        ctx.enter_context(
            nc.allow_low_precision(reason="fp16 matmuls are the design point")
        )
        tc = ctx.enter_context(tile.TileContext(nc))
        const = ctx.enter_context(tc.tile_pool(name="const", bufs=1))
        xp = ctx.enter_context(tc.tile_pool(name="xp", bufs=2))  # 2 bufs x 3 tags
        ptp = ctx.enter_context(tc.tile_pool(name="ptp", bufs=2))
        dnp = ctx.enter_context(tc.tile_pool(name="dnp", bufs=2))
        otp = ctx.enter_context(tc.tile_pool(name="otp", bufs=3))
        # PSUM: sc 4 banks + acc 2 + po 2 = 8
        scp = ctx.enter_context(tc.tile_pool(name="scp", bufs=1, space="PSUM"))
        accp = ctx.enter_context(tc.tile_pool(name="accp", bufs=2, space="PSUM"))
        pop = ctx.enter_context(tc.tile_pool(name="pop", bufs=2, space="PSUM"))

        # ---- constants / persistent tensors ----
        # masks first: the PE warm-up matmuls depend only on this small DMA,
        # so the PE clock ramps while the big loads stream in.
        msk_sb = const.tile([P, KPQ, QCH], F16)
        nc.sync.dma_start(out=msk_sb, in_=msk[:, :, :])
        wq_sb = const.tile([P, ET, F], F16)
        nc.sync.dma_start(out=wq_sb, in_=wq3[:, :, :])
        wk_sb = const.tile([P, ET, F], F16)
        nc.sync.dma_start(out=wk_sb, in_=wk3[:, :, :])
        wv_sb = const.tile([P, ET, F], F16)
        nc.sync.dma_start(out=wv_sb, in_=wv3[:, :, :])
        wo_sb = const.tile([P, F // P, E], F16)
        nc.sync.dma_start(out=wo_sb, in_=wo3[:, :, :])
        bcat_sb = const.tile([P, 2 + 2 + F], FP32)
        nc.sync.dma_start(out=bcat_sb, in_=bcat[:, :])
        bq_sb = bcat_sb[:, 0:2]
        bk_sb = bcat_sb[:, 2:4]
        bvb_sb = bcat_sb[:, 4:4 + F]

        # PE clock warm-up: back-to-back dummy matmuls (WAW-serialized on the
        # acc pool) keep the tensor engine busy through the HAM window while
        # the input DMAs stream, so real work starts at 2.4 GHz.
        for _ in range(14):
            wps = accp.tile([P, QCH], FP32, tag="acc")
            nc.tensor.matmul(
                wps, msk_sb[:, 0, 0:P], msk_sb[:, 0, :], start=True, stop=True
            )

        ones_f32 = const.tile([P, D], FP32)
        nc.vector.memset(ones_f32, 1.0)
        ones_f16 = const.tile([P, D], F16)
        nc.scalar.activation(ones_f16, ones_f32, AF.Copy)

        qT_sb = const.tile([P, F // P, S], F16)
        kT_sb = const.tile([P, F // P, S], F16)
        # V with a trailing ones column: the PV matmul emits the softmax
        # denominator as PSUM row D for free.
        vo_sb = const.tile([P, KT, HL, D + 1], F16)
        nc.scalar.activation(
            vo_sb[:, :, :, D:D + 1],
            ones_f32[:, 0:KT * HL].rearrange("p (a b c) -> p a b c", a=KT, b=HL, c=1),
            AF.Copy,
        )
        oT_sb = const.tile([P, F // P, S], F16)
        # unnormalized attention output + denominator row (row D), per head
        ou_all = const.tile([P, HL, S], F16)

        # x chunk DMAs (j-granular); emitted early and prefetched one chunk
        # ahead by the main loop.
        x_tiles = {}

        def emit_x_dma(j):
            for name, src in (("q", xq4), ("k", xk4), ("v", xv4)):
                t = xp.tile([P, ET, QCH], F16, tag=f"x{name}")
                nc.sync.dma_start(out=t, in_=src[j])
                x_tiles[(name, j)] = t

        # ---- projection / out-projection unit generators (PE fillers) ----
        def proj_qk_unit(j, which, blk):
            xt = x_tiles[(which, j)]
            w_sb = wq_sb if which == "q" else wk_sb
            b_sb = bq_sb if which == "q" else bk_sb
            dst = qT_sb if which == "q" else kT_sb
            acc = accp.tile([P, QCH], FP32, tag="acc")
            for et in range(ET):
                nc.tensor.matmul(
                    acc,
                    w_sb[:, et, ds(blk * P, P)],
                    xt[:, et, :],
                    start=(et == 0),
                    stop=(et == ET - 1),
                )
            nc.vector.tensor_scalar_add(
                dst[:, blk, ds(j * QCH, QCH)], acc, b_sb[:, blk:blk + 1]
            )
            return 1750

        def proj_v_unit(j, sl):
            xt = x_tiles[("v", j)]
            st = j * KPQ + sl
            acc = accp.tile([P, QCH], FP32, tag="acc")
            for et in range(ET):
                nc.tensor.matmul(
                    acc[:, 0:F],
                    xt[:, et, ds(sl * P, P)],
                    wv_sb[:, et, :],
                    start=(et == 0),
                    stop=(et == ET - 1),
                )
            nc.vector.tensor_add(
                vo_sb[:, st, :, 0:D],
                acc[:, 0:F].rearrange("p (h d) -> p h d", h=HL),
                bvb_sb.rearrange("p (h d) -> p h d", h=HL),
            )
            return 900

        def outproj_unit(j, eb):
            acc = accp.tile([P, QCH], FP32, tag="acc")
            for fb in range(F // P):
                nc.tensor.matmul(
                    acc,
                    wo_sb[:, fb, ds(eb * P, P)],
                    oT_sb[:, fb, ds(j * QCH, QCH)],
                    start=(fb == 0),
                    stop=(fb == F // P - 1),
                )
            ot = otp.tile([P, QCH], F16, tag="ot")
            if eb % 2 == 0:
                nc.vector.tensor_copy(ot, acc)
            else:
                nc.scalar.activation(ot, acc, AF.Copy)
            nc.sync.dma_start(out=outT[ds(eb * P, P), ds(j * QCH, QCH)], in_=ot)
            return 500

        fillers = deque()

        def do_filler(budget):
            while budget > 0 and fillers:
                budget -= fillers.popleft()()

        def drain_fillers():
            while fillers:
                fillers.popleft()()

        def push_proj(j):
            for blk in range(F // P):
                fillers.append(lambda j=j, b=blk: proj_qk_unit(j, "q", b))
                fillers.append(lambda j=j, b=blk: proj_qk_unit(j, "k", b))
            for sl in range(KPQ):
                fillers.append(lambda j=j, s=sl: proj_v_unit(j, s))

        def push_outproj(j):
            for eb in range(E // P):
                fillers.append(lambda j=j, e=eb: outproj_unit(j, e))

        if debug:
            dbg_po_sb = const.tile([P, HL, QCH], FP32)
            dbg_bc_sb = const.tile([D, HL, QCH], FP32)

        # ---- normalization ----
        # po (unnormalized O + denom row) is evacuated to SBUF fp16 right
        # after the last PV; per chunk j, ONE tiny DMA reshapes the 4 heads'
        # denominator rows [1, 4x512] into [128, 16] so a single DVE
        # reciprocal covers them at full lane parallelism (~265ns vs 3.4us
        # per single-partition reciprocal), then a DMA puts 1/denom back as
        # a row for the PE broadcast matmuls.
        def emit_evac(j, h, po_t):
            nc.scalar.activation(
                ou_all[0:D + 1, h, ds(j * QCH, QCH)], po_t[0:D + 1, :], AF.Copy
            )
            if debug and j == 0:
                nc.vector.tensor_copy(dbg_po_sb[:, h, :], po_t)

        def emit_norm_j(j):
            PPH = QCH // 16  # 32 partitions per head's denominator row
            dn = dnp.tile([P, 16], F16, tag="dn")
            for h in range(HL):
                nc.sync.dma_start(
                    out=dn[h * PPH:(h + 1) * PPH, :],
                    in_=ou_all[D:D + 1, h, ds(j * QCH, QCH)],
                )
            rc = dnp.tile([P, 16], F16, tag="rc")
            nc.vector.reciprocal(rc, dn)
            rcr = dnp.tile([1, HL, QCH], F16, tag="rcr")
            for h in range(HL):
                nc.sync.dma_start(
                    out=rcr[:, h, :], in_=rc[h * PPH:(h + 1) * PPH, :]
                )
            for h in range(HL):
                blkh = h // 2
                doff = (h % 2) * D
                bc = accp.tile([P, QCH], FP32, tag="acc")
                nc.tensor.matmul(
                    bc[0:D, :], ones_f16[0:1, :], rcr[:, h, :],
                    start=True, stop=True,
                )
                if debug and j == 0:
                    nc.vector.tensor_copy(dbg_bc_sb[:, h, :], bc[0:D, :])
                nc.vector.tensor_mul(
                    oT_sb[doff:doff + D, blkh, ds(j * QCH, QCH)],
                    ou_all[0:D, h, ds(j * QCH, QCH)],
                    bc[0:D, :],
                )

        # ---- main emission loop ----
        emit_x_dma(0)
        push_proj(0)
        drain_fillers()          # projections for chunk 0 up front

        pending = None
        for j in range(NQ):
            if j + 1 < NQ:
                emit_x_dma(j + 1)
                push_proj(j + 1)
            for h in range(HL):
                blkh = h // 2
                doff = (h % 2) * D
                nkt = KPQ * (j + 1) if causal else KT
                ngrp = (nkt + GRP - 1) // GRP
                po_t = pop.tile([P, QCH], FP32, tag="po")
                for g in range(ngrp):
                    kts = list(range(g * GRP, min(nkt, (g + 1) * GRP)))
                    nk = len(kts)
                    sc = scp.tile([P, GRP, QCH], FP32, tag="sc")
                    for i, kt in enumerate(kts):
                        nc.tensor.matmul(
                            sc[:, i, :],
                            kT_sb[doff:doff + D, blkh, ds(kt * P, P)],
                            qT_sb[doff:doff + D, blkh, ds(j * QCH, QCH)],
                            start=True,
                            stop=True,
                        )
                    pt = ptp.tile([P, GRP, QCH], F16, tag="pt")
                    nc.scalar.activation(pt[:, 0:nk, :], sc[:, 0:nk, :], AF.Exp)
                    if causal and kts[-1] >= KPQ * j:
                        # diagonal group: zero the upper-triangular part
                        nc.vector.tensor_mul(
                            pt[:, 0:nk, :], pt[:, 0:nk, :], msk_sb[:, 0:nk, :]
                        )
                    do_filler(1400)
                    for i, kt in enumerate(kts):
                        nc.tensor.matmul(
                            po_t[0:D + 1, :],
                            vo_sb[:, kt, h, :],
                            pt[:, i, :],
                            start=(kt == 0),
                            stop=(kt == nkt - 1),
                        )
                emit_evac(j, h, po_t)
                if h == 0 and pending is not None:
                    emit_norm_j(pending)
                    push_outproj(pending)
                    pending = None
            pending = j
            # chunk boundary: everything for chunk j+1's attention must be
            # emitted before its first scores matmul.
            drain_fillers()
        emit_norm_j(NQ - 1)
        push_outproj(NQ - 1)
        drain_fillers()
        if debug:
            nc.sync.dma_start(out=dbg_qT[:, :, :], in_=qT_sb)
            nc.sync.dma_start(out=dbg_kT[:, :, :], in_=kT_sb)
            nc.sync.dma_start(out=dbg_vo[:, :, :, :], in_=vo_sb)
            nc.sync.dma_start(out=dbg_oT[:, :, :], in_=oT_sb)
            nc.sync.dma_start(out=dbg_po[:, :, :], in_=dbg_po_sb)
            nc.sync.dma_start(out=dbg_bc[:, :, :], in_=dbg_bc_sb)

    nc.compile()
    return nc


def make_masks(S=S_FULL):
    KPQ = QCH // P
    m = np.zeros((P, KPQ, QCH), np.float32)
    for t in range(KPQ):
        kk = np.arange(P)[:, None]
        qq = np.arange(QCH)[None, :]
        m[:, t, :] = (qq >= kk + P * t).astype(np.float32)
    return m


def make_in_maps(query, key, value, Wq, bq, Wk, bk, Wv, bv, Wo, bo, S=S_FULL):
    scale = float(D) ** -0.5
    ET = E // P
    NQ = S // QCH
    q = np.asarray(query, np.float32)
    k = np.asarray(key, np.float32)
    v = np.asarray(value, np.float32)
    Wq = np.asarray(Wq, np.float32)
    Wk = np.asarray(Wk, np.float32)
    Wv = np.asarray(Wv, np.float32)
    Wo = np.asarray(Wo, np.float32)
    bq = np.asarray(bq, np.float32)
    bk = np.asarray(bk, np.float32)
    bv = np.asarray(bv, np.float32)

    def xswiz(xT):
        # [E, S] -> [NQ, P, ET, QCH]: contiguous per-partition DMA streams
        return np.ascontiguousarray(
            xT.reshape(ET, P, NQ, QCH).transpose(2, 1, 0, 3).astype(np.float16)
        )

    def wswiz(wT):
        # [E, F] -> [P, ET, F]
        return np.ascontiguousarray(
            wT.reshape(ET, P, F).transpose(1, 0, 2).astype(np.float16)
        )

    masks = make_masks(S).astype(np.float16)
    in_maps = []
    for c in range(NCORES):
        b, tp = divmod(c, TP)
        rows = slice(tp * F, (tp + 1) * F)
        bq2 = (bq[rows] * scale).reshape(F // P, P).T        # [P, 2]
        bk2 = bk[rows].reshape(F // P, P).T                  # [P, 2]
        bvb = np.broadcast_to(bv[rows], (P, F))              # [P, F]
        bcat = np.concatenate([bq2, bk2, bvb], axis=1).astype(np.float32)
        woT = Wo[:, rows].T                                  # [F, E]
        wo3 = woT.reshape(F // P, P, E).transpose(1, 0, 2).astype(np.float16)
        in_maps.append({
            "xq4": xswiz(q[b].T),
            "xk4": xswiz(k[b].T),
            "xv4": xswiz(v[b].T),
            "wq3": wswiz((Wq[rows] * scale).T),
            "wk3": wswiz(Wk[rows].T),
            "wv3": wswiz(Wv[rows].T),
            "wo3": np.ascontiguousarray(wo3),
            "bcat": np.ascontiguousarray(bcat),
            "msk": masks,
        })
    return in_maps


_CACHE = {}


def _get_nc(causal):
    if causal not in _CACHE:
        _CACHE[causal] = build(S_FULL, causal)
    return _CACHE[causal]


def kernel(query, key, value, Wq, bq, Wk, bk, Wv, bv, Wo, bo, is_causal):
    causal = bool(int(np.asarray(is_causal)))
    nc = _get_nc(causal)
    in_maps = make_in_maps(query, key, value, Wq, bq, Wk, bk, Wv, bv, Wo, bo)
    res = run_bass_kernel_spmd(nc, in_maps, core_ids=list(range(NCORES)))
    out = np.zeros((B, S_FULL, E), np.float32)
    for c in range(NCORES):
        b, tp = divmod(c, TP)
        out[b] += res.results[c]["outT"].T.astype(np.float32)
    out += np.asarray(bo, np.float32)
    return out


# revision 32
# speedup vs baseline: 1.4944x; 1.0521x over previous
"""Multi-head causal attention (QKV proj + attention + out proj) on 8 TRN2
NeuronCores.

Sharding: 2-way data-parallel over batch x 4-way tensor-parallel over heads
(Megatron-style).  Core c handles batch c//4 and heads [4*(c%4), 4*(c%4)+4).
Each core computes its 4 heads' Q/K/V projections (column-parallel), the
attention for those heads, and a partial output projection (row-parallel).
The host sums the 4 TP partials per batch and adds the output bias.

v2 design notes (vs the 333us baseline, which was PE-bound at 1.2 GHz):
  - everything fp16 on the wire: x, weights, output partials are cast on the
    host, halving HBM traffic (30MB -> ~15MB per core).  Host also
    pre-swizzles x/weights into partition-major layout so every input DMA is
    a contiguous per-partition stream.
  - exp batching: scores for 4 k-tiles accumulate into one [128, 2048] PSUM
    region (4 banks) and are exp'd by ONE activation instruction --
    (2048+352)/1.2 = 2.0us per 4 tiles vs 4x720ns.  ACT is the attention
    pace-setter so this matters.
  - normalization: reciprocal_approx_fast (custom DVE op, ~670ns for [1,512])
    replaces the 3355ns iterative reciprocal; the chain is emitted one head
    late so it never blocks the PE stream.
  - software-pipelined emission: the engines are in-order, so the PE
    instruction stream interleaves attention groups of chunk j with
    projection chains of chunk j+1 and out-projection blocks of chunk j-1
    ("filler" units).  The PE never sits on an exp dependency and the HAM
    clock gate stays at 8/8 (2.4 GHz).
"""

import numpy as np
from collections import deque
from contextlib import ExitStack

import concourse.bass as bass
import concourse.mybir as mybir
import concourse.tile as tile
from concourse import bacc
from concourse.bass import ds
from concourse.bass_utils import run_bass_kernel_spmd


B, S_FULL, E, H = 2, 2048, 1024, 16
D = E // H          # 64
NCORES = 8
TP = 4              # tensor-parallel ways (over heads)
HL = H // TP        # 4 local heads per core
F = HL * D          # 256 local projection width
P = 128
QCH = 512           # q-chunk / matmul moving-dim size
GRP = 4             # k-tiles exp'd per activation instruction
FP32 = mybir.dt.float32
F32R = mybir.dt.float32r
F16 = mybir.dt.float16
AF = mybir.ActivationFunctionType


def build(S=S_FULL, causal=True, debug=False):
    ET = E // P          # 8 contraction tiles for projections
    NQ = S // QCH        # 4 q chunks
    KT = S // P          # 16 k tiles
    KPQ = QCH // P       # 4 k tiles per q chunk

    nc = bacc.Bacc()

    def din(name, shape, dt=F16):
        return nc.declare_dram_parameter(name, shape, dt, isOutput=False)

    # host pre-swizzled, all fp16 (see make_in_maps)
    xq4 = din("xq4", [NQ, P, ET, QCH])
    xk4 = din("xk4", [NQ, P, ET, QCH])
    xv4 = din("xv4", [NQ, P, ET, QCH])
    wq3 = din("wq3", [P, ET, F])
    wk3 = din("wk3", [P, ET, F])
    wv3 = din("wv3", [P, ET, F])
    wo3 = din("wo3", [P, F // P, E])
    bcat = din("bcat", [P, 2 + 2 + F], FP32)   # bq2 | bk2 | bvb
    msk = din("msk", [P, KPQ, QCH])
    outT = nc.declare_dram_parameter("outT", [E, S], F16, isOutput=True)
    if debug:
        dbg_qT = nc.declare_dram_parameter("dbg_qT", [P, F // P, S], F16, isOutput=True)
        dbg_kT = nc.declare_dram_parameter("dbg_kT", [P, F // P, S], F16, isOutput=True)
        dbg_vo = nc.declare_dram_parameter("dbg_vo", [P, KT, HL, D + 1], F16, isOutput=True)
        dbg_oT = nc.declare_dram_parameter("dbg_oT", [P, F // P, S], F16, isOutput=True)
        dbg_po = nc.declare_dram_parameter("dbg_po", [P, HL, QCH], FP32, isOutput=True)
        dbg_bc = nc.declare_dram_parameter("dbg_bc", [D, HL, QCH], FP32, isOutput=True)

    with ExitStack() as ctx:
        ctx.enter_context(
            nc.allow_low_precision(reason="fp16 matmuls are the design point")
        )
        tc = ctx.enter_context(tile.TileContext(nc))
        const = ctx.enter_context(tc.tile_pool(name="const", bufs=1))
        xp = ctx.enter_context(tc.tile_pool(name="xp", bufs=2))  # 2 bufs x 3 tags
        ptp = ctx.enter_context(tc.tile_pool(name="ptp", bufs=2))
        dnp = ctx.enter_context(tc.tile_pool(name="dnp", bufs=2))
        otp = ctx.enter_context(tc.tile_pool(name="otp", bufs=3))
        # PSUM: sc 4 banks + acc 2 + po 2 = 8
        scp = ctx.enter_context(tc.tile_pool(name="scp", bufs=1, space="PSUM"))
        accp = ctx.enter_context(tc.tile_pool(name="accp", bufs=2, space="PSUM"))
        pop = ctx.enter_context(tc.tile_pool(name="pop", bufs=2, space="PSUM"))

        # ---- constants / persistent tensors ----
        # masks first: the PE warm-up matmuls depend only on this small DMA,
        # so the PE clock ramps while the big loads stream in.
        msk_sb = const.tile([P, KPQ, QCH], F16)
        nc.sync.dma_start(out=msk_sb, in_=msk[:, :, :])
        wq_sb = const.tile([P, ET, F], F16)
        nc.sync.dma_start(out=wq_sb, in_=wq3[:, :, :])
        wk_sb = const.tile([P, ET, F], F16)
        nc.sync.dma_start(out=wk_sb, in_=wk3[:, :, :])
        bcat_sb = const.tile([P, 2 + 2 + F], FP32)
        nc.sync.dma_start(out=bcat_sb, in_=bcat[:, :])
        bq_sb = bcat_sb[:, 0:2]
        bk_sb = bcat_sb[:, 2:4]
        bvb_sb = bcat_sb[:, 4:4 + F]
        # wv/wo load AFTER the chunk-0 activations (emitted in the main
        # sequence below): they aren't needed until the V projection /
        # out-projection, and this unblocks proj(0)'s QK chains ~3us earlier.
        wv_sb = const.tile([P, ET, F], F16)
        wo_sb = const.tile([P, F // P, E], F16)

        # PE clock warm-up: back-to-back dummy matmuls (WAW-serialized on the
        # acc pool) keep the tensor engine busy through the HAM window while
        # the input DMAs stream, so real work starts at 2.4 GHz.
        for _ in range(14):
            wps = accp.tile([P, QCH], FP32, tag="acc")
            nc.tensor.matmul(
                wps, msk_sb[:, 0, 0:P], msk_sb[:, 0, :], start=True, stop=True
            )

        ones_f32 = const.tile([P, D], FP32)
        nc.vector.memset(ones_f32, 1.0)
        ones_f16 = const.tile([P, D], F16)
        nc.scalar.activation(ones_f16, ones_f32, AF.Copy)

        qT_sb = const.tile([P, F // P, S], F16)
        kT_sb = const.tile([P, F // P, S], F16)
        # V with a trailing ones column: the PV matmul emits the softmax
        # denominator as PSUM row D for free.
        vo_sb = const.tile([P, KT, HL, D + 1], F16)
        nc.scalar.activation(
            vo_sb[:, :, :, D:D + 1],
            ones_f32[:, 0:KT * HL].rearrange("p (a b c) -> p a b c", a=KT, b=HL, c=1),
            AF.Copy,
        )
        oT_sb = const.tile([P, F // P, S], F16)
        # unnormalized attention output + denominator row (row D), per head
        ou_all = const.tile([P, HL, S], F16)

        # x chunk DMAs (j-granular); emitted early and prefetched one chunk
        # ahead by the main loop.
        x_tiles = {}

        def emit_x_dma(j):
            for name, src in (("q", xq4), ("k", xk4), ("v", xv4)):
                t = xp.tile([P, ET, QCH], F16, tag=f"x{name}")
                nc.sync.dma_start(out=t, in_=src[j])
                x_tiles[(name, j)] = t

        # ---- projection / out-projection unit generators (PE fillers) ----
        def proj_qk_unit(j, which, blk):
            xt = x_tiles[(which, j)]
            w_sb = wq_sb if which == "q" else wk_sb
            b_sb = bq_sb if which == "q" else bk_sb
            dst = qT_sb if which == "q" else kT_sb
            acc = accp.tile([P, QCH], FP32, tag="acc")
            for et in range(ET):
                nc.tensor.matmul(
                    acc,
                    w_sb[:, et, ds(blk * P, P)],
                    xt[:, et, :],
                    start=(et == 0),
                    stop=(et == ET - 1),
                )
            nc.vector.tensor_scalar_add(
                dst[:, blk, ds(j * QCH, QCH)], acc, b_sb[:, blk:blk + 1]
            )
            return 1750

        def proj_v_unit(j, sl):
            xt = x_tiles[("v", j)]
            st = j * KPQ + sl
            acc = accp.tile([P, QCH], FP32, tag="acc")
            for et in range(ET):
                nc.tensor.matmul(
                    acc[:, 0:F],
                    xt[:, et, ds(sl * P, P)],
                    wv_sb[:, et, :],
                    start=(et == 0),
                    stop=(et == ET - 1),
                )
            nc.vector.tensor_add(
                vo_sb[:, st, :, 0:D],
                acc[:, 0:F].rearrange("p (h d) -> p h d", h=HL),
                bvb_sb.rearrange("p (h d) -> p h d", h=HL),
            )
            return 900

        def outproj_unit(j, eb):
            acc = accp.tile([P, QCH], FP32, tag="acc")
            for fb in range(F // P):
                nc.tensor.matmul(
                    acc,
                    wo_sb[:, fb, ds(eb * P, P)],
                    oT_sb[:, fb, ds(j * QCH, QCH)],
                    start=(fb == 0),
                    stop=(fb == F // P - 1),
                )
            ot = otp.tile([P, QCH], F16, tag="ot")
            if eb % 2 == 0:
                nc.vector.tensor_copy(ot, acc)
            else:
                nc.scalar.activation(ot, acc, AF.Copy)
            nc.sync.dma_start(out=outT[ds(eb * P, P), ds(j * QCH, QCH)], in_=ot)
            return 500

        fillers = deque()

        def do_filler(budget):
            while budget > 0 and fillers:
                budget -= fillers.popleft()()

        def drain_fillers():
            while fillers:
                fillers.popleft()()

        def push_proj(j):
            for blk in range(F // P):
                fillers.append(lambda j=j, b=blk: proj_qk_unit(j, "q", b))
                fillers.append(lambda j=j, b=blk: proj_qk_unit(j, "k", b))
            for sl in range(KPQ):
                fillers.append(lambda j=j, s=sl: proj_v_unit(j, s))

        def push_outproj(j):
            for eb in range(E // P):
                fillers.append(lambda j=j, e=eb: outproj_unit(j, e))

        if debug:
            dbg_po_sb = const.tile([P, HL, QCH], FP32)
            dbg_bc_sb = const.tile([D, HL, QCH], FP32)

        # ---- normalization ----
        # po (unnormalized O + denom row) is evacuated to SBUF fp16 right
        # after the last PV; per chunk j, ONE tiny DMA reshapes the 4 heads'
        # denominator rows [1, 4x512] into [128, 16] so a single DVE
        # reciprocal covers them at full lane parallelism (~265ns vs 3.4us
        # per single-partition reciprocal), then a DMA puts 1/denom back as
        # a row for the PE broadcast matmuls.
        def emit_evac(j, h, po_t):
            nc.scalar.activation(
                ou_all[0:D + 1, h, ds(j * QCH, QCH)], po_t[0:D + 1, :], AF.Copy
            )
            if debug and j == 0:
                nc.vector.tensor_copy(dbg_po_sb[:, h, :], po_t)

        def emit_norm_j(j):
            PPH = QCH // 16  # 32 partitions per head's denominator row
            dn = dnp.tile([P, 16], F16, tag="dn")
            for h in range(HL):
                nc.sync.dma_start(
                    out=dn[h * PPH:(h + 1) * PPH, :],
                    in_=ou_all[D:D + 1, h, ds(j * QCH, QCH)],
                )
            rc = dnp.tile([P, 16], F16, tag="rc")
            nc.vector.reciprocal(rc, dn)
            rcr = dnp.tile([1, HL, QCH], F16, tag="rcr")
            for h in range(HL):
                nc.sync.dma_start(
                    out=rcr[:, h, :], in_=rc[h * PPH:(h + 1) * PPH, :]
                )
            for h in range(HL):
                blkh = h // 2
                doff = (h % 2) * D
                bc = accp.tile([P, QCH], FP32, tag="acc")
                nc.tensor.matmul(
                    bc[0:D, :], ones_f16[0:1, :], rcr[:, h, :],
                    start=True, stop=True,
                )
                if debug and j == 0:
                    nc.vector.tensor_copy(dbg_bc_sb[:, h, :], bc[0:D, :])
                nc.vector.tensor_mul(
                    oT_sb[doff:doff + D, blkh, ds(j * QCH, QCH)],
                    ou_all[0:D, h, ds(j * QCH, QCH)],
                    bc[0:D, :],
                )

        # ---- main emission loop ----
        emit_x_dma(0)
        nc.sync.dma_start(out=wv_sb, in_=wv3[:, :, :])
        nc.sync.dma_start(out=wo_sb, in_=wo3[:, :, :])
        push_proj(0)
        drain_fillers()          # projections for chunk 0 up front

        pending = None
        for j in range(NQ):
            if j + 1 < NQ:
                emit_x_dma(j + 1)
                push_proj(j + 1)
            for pr in range(HL // 2):
                # head pair (hA, hB) = (2*pr, 2*pr+1): hA's Q/K live on
                # partitions 0-63 of block pr, hB's on 64-127.  Their QK^T
                # matmuls (64-row contraction each) are emitted back-to-back
                # with explicit tile_position so they stream CONCURRENTLY
                # through disjoint PE row groups -- ~2x scores throughput.
                hA, hB = 2 * pr, 2 * pr + 1
                nkt = KPQ * (j + 1) if causal else KT
                ngrp = nkt // 2          # 2 k-tiles per head per group
                po_a = pop.tile([P, QCH], FP32, tag="po")
                po_b = pop.tile([P, QCH], FP32, tag="po")
                for g in range(ngrp):
                    kts = (2 * g, 2 * g + 1)
                    sc = scp.tile([P, GRP, QCH], FP32, tag="sc")
                    for i, kt in enumerate(kts):
                        nc.tensor.matmul(
                            sc[:, i, :],
                            kT_sb[0:D, pr, ds(kt * P, P)],
                            qT_sb[0:D, pr, ds(j * QCH, QCH)],
                            start=True, stop=True,
                            tile_position=(0, 0),
                        )
                        nc.tensor.matmul(
                            sc[:, 2 + i, :],
                            kT_sb[D:P, pr, ds(kt * P, P)],
                            qT_sb[D:P, pr, ds(j * QCH, QCH)],
                            start=True, stop=True,
                            tile_position=(64, 0),
                        )
                    pt = ptp.tile([P, GRP, QCH], F16, tag="pt")
                    is_diag = causal and kts[-1] >= KPQ * j
                    if is_diag and j >= 2:
                        # late chunks are ACT-bound: compute this group's exp
                        # on the DVE instead, via the fp16 bit-trick
                        # exp(x) ~= bitcast_f16(int16(x*1024/ln2 + 15360)).
                        # Valid for x in (-10.4, 10.6); scores here are ~+-3.
                        nc.vector.tensor_scalar(
                            pt.bitcast(mybir.dt.int16),
                            sc,
                            1477.3194,
                            15360.0,
                            op0=mybir.AluOpType.mult,
                            op1=mybir.AluOpType.add,
                        )
                    else:
                        nc.scalar.activation(pt, sc, AF.Exp)
                    if is_diag:
                        # diagonal group: zero the upper-triangular part
                        toff = 2 * g - KPQ * j   # mask slot of kts[0]
                        nc.vector.tensor_mul(
                            pt[:, 0:2, :], pt[:, 0:2, :],
                            msk_sb[:, toff:toff + 2, :],
                        )
                        nc.vector.tensor_mul(
                            pt[:, 2:4, :], pt[:, 2:4, :],
                            msk_sb[:, toff:toff + 2, :],
                        )
                    do_filler(1400)
                    for i, kt in enumerate(kts):
                        nc.tensor.matmul(
                            po_a[0:D + 1, :],
                            vo_sb[:, kt, hA, :],
                            pt[:, i, :],
                            start=(kt == 0),
                            stop=(kt == nkt - 1),
                        )
                        nc.tensor.matmul(
                            po_b[0:D + 1, :],
                            vo_sb[:, kt, hB, :],
                            pt[:, 2 + i, :],
                            start=(kt == 0),
                            stop=(kt == nkt - 1),
                        )
                emit_evac(j, hA, po_a)
                emit_evac(j, hB, po_b)
                if pr == 0 and pending is not None:
                    emit_norm_j(pending)
                    push_outproj(pending)
                    pending = None
            pending = j
            # chunk boundary: everything for chunk j+1's attention must be
            # emitted before its first scores matmul.
            drain_fillers()
        emit_norm_j(NQ - 1)
        push_outproj(NQ - 1)
        drain_fillers()
        if debug:
            nc.sync.dma_start(out=dbg_qT[:, :, :], in_=qT_sb)
            nc.sync.dma_start(out=dbg_kT[:, :, :], in_=kT_sb)
            nc.sync.dma_start(out=dbg_vo[:, :, :, :], in_=vo_sb)
            nc.sync.dma_start(out=dbg_oT[:, :, :], in_=oT_sb)
            nc.sync.dma_start(out=dbg_po[:, :, :], in_=dbg_po_sb)
            nc.sync.dma_start(out=dbg_bc[:, :, :], in_=dbg_bc_sb)

    nc.compile()
    return nc


def make_masks(S=S_FULL):
    KPQ = QCH // P
    m = np.zeros((P, KPQ, QCH), np.float32)
    for t in range(KPQ):
        kk = np.arange(P)[:, None]
        qq = np.arange(QCH)[None, :]
        m[:, t, :] = (qq >= kk + P * t).astype(np.float32)
    return m


def make_in_maps(query, key, value, Wq, bq, Wk, bk, Wv, bv, Wo, bo, S=S_FULL):
    scale = float(D) ** -0.5
    ET = E // P
    NQ = S // QCH
    q = np.asarray(query, np.float32)
    k = np.asarray(key, np.float32)
    v = np.asarray(value, np.float32)
    Wq = np.asarray(Wq, np.float32)
    Wk = np.asarray(Wk, np.float32)
    Wv = np.asarray(Wv, np.float32)
    Wo = np.asarray(Wo, np.float32)
    bq = np.asarray(bq, np.float32)
    bk = np.asarray(bk, np.float32)
    bv = np.asarray(bv, np.float32)

    def xswiz(xT):
        # [E, S] -> [NQ, P, ET, QCH]: contiguous per-partition DMA streams
        return np.ascontiguousarray(
            xT.reshape(ET, P, NQ, QCH).transpose(2, 1, 0, 3).astype(np.float16)
        )

    def wswiz(wT):
        # [E, F] -> [P, ET, F]
        return np.ascontiguousarray(
            wT.reshape(ET, P, F).transpose(1, 0, 2).astype(np.float16)
        )

    masks = make_masks(S).astype(np.float16)
    in_maps = []
    for c in range(NCORES):
        b, tp = divmod(c, TP)
        rows = slice(tp * F, (tp + 1) * F)
        bq2 = (bq[rows] * scale).reshape(F // P, P).T        # [P, 2]
        bk2 = bk[rows].reshape(F // P, P).T                  # [P, 2]
        bvb = np.broadcast_to(bv[rows], (P, F))              # [P, F]
        bcat = np.concatenate([bq2, bk2, bvb], axis=1).astype(np.float32)
        woT = Wo[:, rows].T                                  # [F, E]
        wo3 = woT.reshape(F // P, P, E).transpose(1, 0, 2).astype(np.float16)
        in_maps.append({
            "xq4": xswiz(q[b].T),
            "xk4": xswiz(k[b].T),
            "xv4": xswiz(v[b].T),
            "wq3": wswiz((Wq[rows] * scale).T),
            "wk3": wswiz(Wk[rows].T),
            "wv3": wswiz(Wv[rows].T),
            "wo3": np.ascontiguousarray(wo3),
            "bcat": np.ascontiguousarray(bcat),
            "msk": masks,
        })
    return in_maps


_CACHE = {}


def _get_nc(causal):
    if causal not in _CACHE:
        _CACHE[causal] = build(S_FULL, causal)
    return _CACHE[causal]


def kernel(query, key, value, Wq, bq, Wk, bk, Wv, bv, Wo, bo, is_causal):
    causal = bool(int(np.asarray(is_causal)))
    nc = _get_nc(causal)
    in_maps = make_in_maps(query, key, value, Wq, bq, Wk, bk, Wv, bv, Wo, bo)
    res = run_bass_kernel_spmd(nc, in_maps, core_ids=list(range(NCORES)))
    out = np.zeros((B, S_FULL, E), np.float32)
    for c in range(NCORES):
        b, tp = divmod(c, TP)
        out[b] += res.results[c]["outT"].T.astype(np.float32)
    out += np.asarray(bo, np.float32)
    return out
